# revision 61
# baseline (speedup 1.0000x reference)
"""2-layer GCN (GCNConv -> ReLU -> GCNConv) on 8 TRN2 NeuronCores.

Sharding: output nodes are split into 8 shards (one per core); edges are
partitioned by destination shard so each core owns the scatter-add for its
nodes. Hidden features of source nodes are exchanged with an on-device
AllGather (bf16 table) between the per-shard transform and the aggregation.

Aggregation dataflow (the bottleneck is SWDGE descriptor generation on the
Pool engine at ~8 ns/descriptor, one descriptor per edge):
  - The shared table is AllGathered as TWO half-shard tables (split at
    block 25, via separate zero-offset staging tensors - AP-offset
    collective inputs corrupt data on HW). Each half-table has < 32768
    rows so int16 gather indices cover it directly, and the first half's
    collective fires mid-transform, overlapping the AllGather with the
    rest of the transform and the first gathers.
  - Edges are pre-sorted into 128-dst-node blocks (49 per core, balanced by
    in-degree via host-side relabeling). Blocks are grouped 4 at a time;
    within a group all blocks' half-A chunks come first, then all half-B
    chunks, so gather calls of 8x128 indices run across block boundaries
    and the per-call fixed cost (~1 us) is amortized, while a block's
    chunks stay within a small window of the call stream.
  - Each dma_gather call lands in its own rotating pool tile (NBUF deep);
    a one-hot matrix S (VectorE iota==dmat compare, bf16 out) folds each
    128-edge chunk into the block's PSUM accumulator on TensorE.
  - The table (AllGathered hidden features) is bf16: halves the AllGather
    and the gather read traffic. Layer-1's x@W1 also runs in bf16.
  - Layer-2's transform (transpose + W2 matmul) is fused into layer-1's
    per-block epilogue so PE/DVE work overlaps Pool descriptor generation.
  - GCN_PREP=1 selects an experimental prepare_only+trigger_dma pipeline
    (descriptor gen ahead of the AllGather). With explicit consumer waits
    it is numerically correct but ~1.2 ms slower (per-call prep/trigger
    overhead), so the default stays the direct gather path.
"""

import os

import numpy as np

P = 128
N_CORES = 8
N_NODES = 50000
IN_DIM = 256
HID = 128
NB = 49
SHARD = NB * P  # 6272
NPAD = N_CORES * SHARD  # 50176
# The shared table is AllGathered in two halves (split at block 25 of each
# shard) so each half-table has < 32768 rows and int16 gather indices cover
# it without base tricks, and the first half's collective can fire while
# the second half is still being produced.
HALF_A = 25 * P  # 3200 rows per shard
HALF_B = SHARD - HALF_A  # 3072
NPAD_A = N_CORES * HALF_A  # 25600
NPAD_B = N_CORES * HALF_B  # 24576
CALL_CHUNKS = 8  # 1024 indices per dma_gather call (SWDGE per-call limit)
NG = 96  # max chunk-span constraint for call construction
PREP_W = int(os.environ.get("GCN_PREP_W", "3"))  # prepare-ahead window (calls)

LAST_EXEC_NS = None


def _wrap16(flat, ncols):
    w = np.zeros((16, ncols), np.uint16)
    n = len(flat)
    w[np.arange(n) % 16, np.arange(n) // 16] = flat
    return np.tile(w, (8, 1)).view(np.int16)


def _host_prep(x, edge_index, W1, b1, W2, b2, table_bf16=True):
    import ml_dtypes

    src = np.asarray(edge_index[0], dtype=np.int64)
    dst = np.asarray(edge_index[1], dtype=np.int64)
    x = np.asarray(x, dtype=np.float32)

    indeg = np.bincount(dst, minlength=N_NODES)
    deg = indeg + 1.0
    dinv = (1.0 / np.sqrt(deg)).astype(np.float32)

    # per-shard relabeling: deal nodes (by in-degree desc) round-robin into
    # the 49 dst blocks so block edge counts are balanced across cores.
    old_shard = N_NODES // N_CORES
    new_of_old = np.empty(N_NODES, np.int64)
    old_of_new = np.full(NPAD, -1, np.int64)
    for c in range(N_CORES):
        olds = np.arange(c * old_shard, (c + 1) * old_shard)
        order = olds[np.argsort(-indeg[olds], kind="stable")]
        pos_in_block = np.arange(len(order)) // NB
        block = np.arange(len(order)) % NB
        news = c * SHARD + block * P + pos_in_block
        new_of_old[order] = news
        old_of_new[news] = order

    src_n = new_of_old[src]
    dst_n = new_of_old[dst]

    core_of_dst = dst_n // SHARD
    lo_lists = [[None] * NB for _ in range(N_CORES)]  # half-A edges
    hi_lists = [[None] * NB for _ in range(N_CORES)]  # half-B edges
    for c in range(N_CORES):
        m = core_of_dst == c
        s, d = src_n[m], dst_n[m] - c * SHARD
        b = d // P
        r = d % P
        c_src = s // SHARD
        r_src = s % SHARD
        isb = r_src >= HALF_A
        idx_a = c_src * HALF_A + r_src
        idx_b = c_src * HALF_B + (r_src - HALF_A)
        for bb in range(NB):
            mb = b == bb
            mlo = mb & ~isb
            mhi = mb & isb
            lo_lists[c][bb] = (idx_a[mlo], r[mlo])
            hi_lists[c][bb] = (idx_b[mhi], r[mhi])

    C_lo = np.zeros(NB, np.int64)
    C_hi = np.zeros(NB, np.int64)
    for b in range(NB):
        for c in range(N_CORES):
            C_lo[b] = max(C_lo[b], (len(lo_lists[c][b][0]) + P - 1) // P)
            C_hi[b] = max(C_hi[b], (len(hi_lists[c][b][0]) + P - 1) // P)

    # Chunk layout: blocks grouped GROUPP at a time; within a group all lo
    # chunks first, then all hi chunks. Gather calls merge across blocks
    # within a run (same table base), and a block's chunks stay within a
    # ~GROUPP*17-chunk window so the NG-slot arena ring can't deadlock.
    GROUPP = 4
    lo_start = np.zeros(NB, np.int64)
    hi_start = np.zeros(NB, np.int64)
    regions = []  # (start_chunk, end_chunk, is_hi)
    at = 0
    for g0 in range(0, NB, GROUPP):
        blks = range(g0, min(g0 + GROUPP, NB))
        r0 = at
        for b in blks:
            lo_start[b] = at
            at += int(C_lo[b])
        regions.append((r0, at, False))
        r0 = at
        for b in blks:
            hi_start[b] = at
            at += int(C_hi[b])
        regions.append((r0, at, True))
    NC = at

    idx_mats, d_mats = [], []
    for c in range(N_CORES):
        idx_flat = np.zeros(NC * P, np.int64)
        dloc = np.full((P, NC), -1.0, np.float32)
        for b in range(NB):
            for lists, start in ((lo_lists, lo_start[b]), (hi_lists, hi_start[b])):
                s, r = lists[c][b]
                n = len(s)
                base = int(start) * P
                idx_flat[base : base + n] = s
                j = np.arange(n)
                dloc[j % P, int(start) + j // P] = r
        idx_mats.append(_wrap16(idx_flat, NC * 8))
        d_mats.append(dloc)

    # calls: greedy 8-chunk runs within each region, never wrapping the
    # NG-slot arena mid-call.
    calls = []
    for r0, r1, is_hi in regions:
        if r1 == r0:
            continue
        at = r0
        while at < r1:
            k = min(CALL_CHUNKS, r1 - at, NG - (at % NG))
            calls.append((at, k, is_hi))
            at += k

    xs, dinvs = [], []
    for c in range(N_CORES):
        xc = np.zeros((SHARD, IN_DIM), np.float32)
        dc = np.ones((SHARD,), np.float32)
        sel = old_of_new[c * SHARD : (c + 1) * SHARD]
        real = sel >= 0
        xc[real] = x[sel[real]]
        dc[real] = dinv[sel[real]]
        dw = dc.reshape(NB, P).T.copy()
        xT = np.ascontiguousarray(xc.T.reshape(2, P, SHARD).transpose(1, 0, 2))
        xs.append(xT.reshape(P, 2 * SHARD).astype(ml_dtypes.bfloat16))
        dinvs.append(dw)

    iota = np.tile(np.arange(P, dtype=np.float32)[None, :], (P, 1))
    ident = np.eye(P, dtype=np.float32)
    b1r = np.tile(np.asarray(b1, np.float32)[None, :], (P, 1))
    b2r = np.tile(np.asarray(b2, np.float32)[None, :], (P, 1))

    in_maps = []
    for c in range(N_CORES):
        in_maps.append(
            {
                "x": xs[c],
                "gidx": idx_mats[c],
                "dmat": d_mats[c],
                "dinv": dinvs[c],
                "w1": np.asarray(W1, np.float32).astype(ml_dtypes.bfloat16),
                "w2": np.asarray(W2, np.float32),
                "b1r": b1r,
                "b2r": b2r,
                "iota": iota,
                "ident": ident,
            }
        )

    meta = dict(
        C_lo=C_lo,
        C_hi=C_hi,
        lo_start=lo_start,
        hi_start=hi_start,
        NC=NC,
        calls=calls,
        old_of_new=old_of_new,
    )
    return in_maps, meta


def _build_program(meta, use_prep=True, table_bf16=True):
    import concourse.mybir as mybir
    import concourse.tile as tile
    from concourse import bacc
    from concourse._compat import get_trn_type

    C_lo = meta["C_lo"]
    C_hi = meta["C_hi"]
    lo_start = meta["lo_start"]
    hi_start = meta["hi_start"]
    NC = meta["NC"]
    calls = meta["calls"]
    f32 = mybir.dt.float32
    bf16 = mybir.dt.bfloat16 if table_bf16 else mybir.dt.float32

    scratch = int(os.environ.get("GCN_SCRATCH", "16384"))
    nc = bacc.Bacc(get_trn_type() or "TRN2", dynamic_dma_scratch_size=scratch)
    x_in = nc.dram_tensor("x", [P, 2 * SHARD], mybir.dt.bfloat16, kind="ExternalInput")
    gidx = nc.dram_tensor("gidx", [P, NC * 8], mybir.dt.int16, kind="ExternalInput")
    dmat = nc.dram_tensor("dmat", [P, NC], f32, kind="ExternalInput")
    dinv_in = nc.dram_tensor("dinv", [P, NB], f32, kind="ExternalInput")
    w1_in = nc.dram_tensor("w1", [IN_DIM, HID], mybir.dt.bfloat16, kind="ExternalInput")
    w2_in = nc.dram_tensor("w2", [HID, HID], f32, kind="ExternalInput")
    b1_in = nc.dram_tensor("b1r", [P, HID], f32, kind="ExternalInput")
    b2_in = nc.dram_tensor("b2r", [P, HID], f32, kind="ExternalInput")
    iota_in = nc.dram_tensor("iota", [P, P], f32, kind="ExternalInput")
    ident_in = nc.dram_tensor("ident", [P, P], f32, kind="ExternalInput")
    z_out = nc.dram_tensor("z", [SHARD, HID], f32, kind="ExternalOutput")

    cc1a_in = nc.dram_tensor("cc1a_in", [HALF_A, HID], bf16)
    cc1b_in = nc.dram_tensor("cc1b_in", [HALF_B, HID], bf16)
    table1a = nc.dram_tensor("table1a", [NPAD_A, HID], bf16, addr_space="Shared")
    table1b = nc.dram_tensor("table1b", [NPAD_B, HID], bf16, addr_space="Shared")
    cc2a_in = nc.dram_tensor("cc2a_in", [HALF_A, HID], bf16)
    cc2b_in = nc.dram_tensor("cc2b_in", [HALF_B, HID], bf16)
    table2a = nc.dram_tensor("table2a", [NPAD_A, HID], bf16, addr_space="Shared")
    table2b = nc.dram_tensor("table2b", [NPAD_B, HID], bf16, addr_space="Shared")

    rg = [list(range(N_CORES))]

    NBUF = 14 if table_bf16 else 10
    with tile.TileContext(nc) as tc:
        with (
            tc.tile_pool(name="persist", bufs=1) as pp,
            tc.tile_pool(name="xt", bufs=4) as xtp,
            tc.tile_pool(name="s", bufs=6) as sp,
            tc.tile_pool(name="ep", bufs=4) as ep,
            tc.tile_pool(name="g", bufs=NBUF) as gp,
            tc.tile_pool(name="psum", bufs=2, space="PSUM") as psp,
        ):
            idx_t = pp.tile([P, NC * 8], mybir.dt.int16)
            nc.sync.dma_start(out=idx_t[:], in_=gidx[:])
            dm_t = pp.tile([P, NC], f32)
            nc.sync.dma_start(out=dm_t[:], in_=dmat[:])
            dinv_t = pp.tile([P, NB], f32)
            nc.sync.dma_start(out=dinv_t[:], in_=dinv_in[:])
            iota_t = pp.tile([P, P], f32)
            nc.sync.dma_start(out=iota_t[:], in_=iota_in[:])
            ident_t = pp.tile([P, P], f32)
            nc.sync.dma_start(out=ident_t[:], in_=ident_in[:])
            b1_t = pp.tile([P, HID], f32)
            nc.sync.dma_start(out=b1_t[:], in_=b1_in[:])
            b2_t = pp.tile([P, HID], f32)
            nc.sync.dma_start(out=b2_t[:], in_=b2_in[:])
            w1_t = pp.tile([P, 2 * HID], mybir.dt.bfloat16)
            nc.sync.dma_start(
                out=w1_t[:].rearrange("p (k h) -> p k h", k=2),
                in_=w1_in[:].rearrange("(k p) h -> p k h", p=P),
            )
            w2_t = pp.tile([P, HID], f32)
            nc.sync.dma_start(out=w2_t[:], in_=w2_in[:])

            hbuf = pp.tile([P, SHARD], f32)
            h2buf = pp.tile([P, SHARD], f32)

            dma_sem = nc.alloc_semaphore("gather_dma")

            # chunk -> (call index, call start chunk)
            call_of_chunk = {}
            for j, (at, k, is_hi) in enumerate(calls):
                for c in range(at, at + k):
                    call_of_chunk[c] = (j, at)

            gtiles = {}
            fired_base = [0]  # calls fired in earlier layers

            def emit_preps(tab_a, tab_b, prep, j0=0, j1=None):
                for j, (at, k, is_b) in list(enumerate(calls))[j0:j1]:
                    n = k * P
                    src = tab_b[:, :] if is_b else tab_a[:, :]
                    G = gp.tile([P, CALL_CHUNKS * P], bf16, tag="g")
                    gtiles[j] = G[:].rearrange("p (c d) -> p c d", d=P)
                    kwargs = dict(prepare_only=True, sem=dma_sem) if prep else {}
                    nc.gpsimd.dma_gather(
                        gtiles[j][:, 0:k, :],
                        src,
                        idx_t[:, at * 8 : (at + k) * 8],
                        n,
                        n,
                        HID,
                        **kwargs,
                    )
                    if prep and j >= PREP_W - 1:
                        nc.gpsimd.trigger_dma(count=None)
                if prep and len(calls) < PREP_W:
                    nc.gpsimd.trigger_dma(count=None)

            def transform2_block(b, x2_tile):
                # fused layer-2 transform for dst block b: h2 = dinv*(x2 @ W2)
                tp = psp.tile([P, P], f32, tag="tp")
                nc.tensor.transpose(out=tp[:], in_=x2_tile, identity=ident_t[:])
                xT = xtp.tile([P, P], f32, tag="xT")
                nc.scalar.copy(out=xT[:], in_=tp[:])
                hp = psp.tile([P, HID], f32, tag="hp2")
                nc.tensor.matmul(
                    out=hp[:], lhsT=xT[:], rhs=w2_t[:], start=True, stop=True
                )
                sl = h2buf[:, b * P : (b + 1) * P]
                nc.vector.tensor_scalar(
                    out=sl, in0=hp[:], scalar1=dinv_t[:, b : b + 1],
                    scalar2=None, op0=mybir.AluOpType.mult,
                )
                hc = xtp.tile([P, HID], bf16, tag="hc2")
                nc.scalar.copy(out=hc[:], in_=sl)
                if b < HALF_A // P:
                    nc.sync.dma_start(
                        out=cc2a_in[b * P : (b + 1) * P, :], in_=hc[:]
                    )
                else:
                    bo = b - HALF_A // P
                    nc.sync.dma_start(
                        out=cc2b_in[bo * P : (bo + 1) * P, :], in_=hc[:]
                    )

            fuse = bool(int(os.environ.get("GCN_FUSE", "1")))
            x2buf = None if fuse else pp.tile([P, SHARD], f32)

            def consume(hsrc, bias_t, relu, z_dram, use_prep_waits=False,
                        b0=0, b1=NB):
                maxj = [-1]

                def need_call(j):
                    # prep path: Tile's consumer waits don't cover the
                    # deferred DMA; wait explicitly on the shared DMA sem.
                    # Completions are FIFO, so >=16*(global call idx+1)
                    # implies all earlier calls landed too.
                    if use_prep_waits and j > maxj[0]:
                        nc.tensor.wait_ge(
                            dma_sem, 16 * (fired_base[0] + j + 1)
                        )
                        maxj[0] = j

                for b in range(b0, b1):
                    cl = [int(lo_start[b]) + i for i in range(int(C_lo[b]))] + [
                        int(hi_start[b]) + i for i in range(int(C_hi[b]))
                    ]
                    acc = psp.tile([P, HID], f32, tag="acc")
                    for i, c in enumerate(cl):
                        S = sp.tile([P, P], bf16, tag="S")
                        nc.vector.tensor_tensor(
                            out=S[:], in0=iota_t[:],
                            in1=dm_t[:, c : c + 1].to_broadcast([P, P]),
                            op=mybir.AluOpType.is_equal,
                        )
                        j, at = call_of_chunk[c]
                        need_call(j)
                        nc.tensor.matmul(
                            out=acc[:], lhsT=S[:], rhs=gtiles[j][:, c - at, :],
                            start=(i == 0), stop=(i == len(cl) - 1),
                        )
                    t1 = ep.tile([P, HID], f32, tag="t1")
                    nc.vector.tensor_tensor(
                        out=t1[:], in0=acc[:],
                        in1=hsrc[:, b * P : (b + 1) * P],
                        op=mybir.AluOpType.add,
                    )
                    t2 = ep.tile([P, HID], f32, tag="t2")
                    nc.vector.scalar_tensor_tensor(
                        out=t2[:], in0=t1[:],
                        scalar=dinv_t[:, b : b + 1], in1=bias_t[:],
                        op0=mybir.AluOpType.mult, op1=mybir.AluOpType.add,
                    )
                    if relu:
                        if fuse:
                            x2 = ep.tile([P, HID], f32, tag="x2")
                            nc.scalar.activation(
                                out=x2[:], in_=t2[:],
                                func=mybir.ActivationFunctionType.Relu,
                            )
                            transform2_block(b, x2[:])
                        else:
                            nc.scalar.activation(
                                out=x2buf[:, b * P : (b + 1) * P], in_=t2[:],
                                func=mybir.ActivationFunctionType.Relu,
                            )
                    else:
                        nc.sync.dma_start(
                            out=z_dram[b * P : (b + 1) * P, :], in_=t2[:]
                        )

            # layer 1 transform: h1 = dinv * (x @ W1), bf16 inputs
            xT_t = pp.tile([P, 2 * SHARD], mybir.dt.bfloat16)
            nc.sync.dma_start(out=xT_t[:], in_=x_in[:])
            xT3 = xT_t[:].rearrange("p (k n) -> p k n", k=2)
            for t in range(NB):
                hp = psp.tile([P, HID], f32, tag="hp")
                for k in range(2):
                    nc.tensor.matmul(
                        out=hp[:], lhsT=xT3[:, k, t * P : (t + 1) * P],
                        rhs=w1_t[:, k * HID : (k + 1) * HID],
                        start=(k == 0), stop=(k == 1),
                    )
                sl = hbuf[:, t * P : (t + 1) * P]
                nc.vector.tensor_scalar(
                    out=sl, in0=hp[:], scalar1=dinv_t[:, t : t + 1],
                    scalar2=None, op0=mybir.AluOpType.mult,
                )
                hc = xtp.tile([P, HID], bf16, tag="hc1")
                nc.scalar.copy(out=hc[:], in_=sl)
                if t < HALF_A // P:
                    nc.sync.dma_start(
                        out=cc1a_in[t * P : (t + 1) * P, :], in_=hc[:]
                    )
                else:
                    to = t - HALF_A // P
                    nc.sync.dma_start(
                        out=cc1b_in[to * P : (to + 1) * P, :], in_=hc[:]
                    )
                if t == HALF_A // P - 1:
                    # first-half table ships while the rest is produced
                    nc.gpsimd.collective_compute(
                        "AllGather", mybir.AluOpType.bypass, replica_groups=rg,
                        ins=[cc1a_in[:]], outs=[table1a[:]],
                    )

            nc.gpsimd.collective_compute(
                "AllGather", mybir.AluOpType.bypass, replica_groups=rg,
                ins=[cc1b_in[:]], outs=[table1b[:]],
            )
            if fuse and not use_prep:
                # Interleaved emission: cc2a sits mid-way through layer-1's
                # Pool gather stream (after group 8's calls, once consume of
                # blocks 0..27 has produced cc2a_in), so AllGather 2a runs
                # under the remaining layer-1 descriptor generation.
                split_call = next(
                    j for j, (at, k, ib) in enumerate(calls)
                    if at >= int(lo_start[36])
                )
                emit_preps(table1a, table1b, False, 0, split_call)
                consume(hbuf, b1_t, True, None, b0=0, b1=28)
                nc.gpsimd.collective_compute(
                    "AllGather", mybir.AluOpType.bypass, replica_groups=rg,
                    ins=[cc2a_in[:]], outs=[table2a[:]],
                )
                emit_preps(table1a, table1b, False, split_call, None)
                consume(hbuf, b1_t, True, None, b0=28, b1=NB)
            else:
                emit_preps(table1a, table1b, use_prep)
                consume(hbuf, b1_t, True, None, use_prep_waits=use_prep)
                if use_prep:
                    fired_base[0] += len(calls)
                if not fuse:
                    for b in range(NB):
                        transform2_block(b, x2buf[:, b * P : (b + 1) * P])
                nc.gpsimd.collective_compute(
                    "AllGather", mybir.AluOpType.bypass, replica_groups=rg,
                    ins=[cc2a_in[:]], outs=[table2a[:]],
                )

            nc.gpsimd.collective_compute(
                "AllGather", mybir.AluOpType.bypass, replica_groups=rg,
                ins=[cc2b_in[:]], outs=[table2b[:]],
            )
            emit_preps(table2a, table2b, use_prep)
            consume(h2buf, b2_t, False, z_out, use_prep_waits=use_prep)

    nc.compile()
    return nc


def kernel(x, edge_index, W1, b1, W2, b2):
    global LAST_EXEC_NS
    from concourse.bass_utils import run_bass_kernel_spmd

    trace = bool(int(os.environ.get("GCN_TRACE", "0")))
    if trace:
        try:  # NTFF profiling shim (axon images lack antenv.axon_hooks)
            _install_ntff_shim()
        except Exception:
            trace = False

    use_prep = bool(int(os.environ.get("GCN_PREP", "0")))
    table_bf16 = bool(int(os.environ.get("GCN_BF16", "1")))
    in_maps, meta = _host_prep(x, edge_index, W1, b1, W2, b2, table_bf16=table_bf16)
    nc = _build_program(meta, use_prep=use_prep, table_bf16=table_bf16)
    res = run_bass_kernel_spmd(
        nc, in_maps, core_ids=list(range(N_CORES)), trace=trace
    )
    LAST_EXEC_NS = res.exec_time_ns

    old_of_new = meta["old_of_new"]
    z = np.zeros((N_NODES, HID), np.float32)
    for c in range(N_CORES):
        zc = np.asarray(res.results[c]["z"])
        sel = old_of_new[c * SHARD : (c + 1) * SHARD]
        real = sel >= 0
        z[sel[real]] = zc[real]
    return z


def _install_ntff_shim():
    import contextlib
    import ctypes
    import sys
    import types

    if "antenv.axon_hooks" in sys.modules:
        return
    lib = ctypes.CDLL("/opt/axon/libaxon_pjrt.so")
    if not hasattr(lib, "axon_start_nrt_profile"):
        raise RuntimeError("no profile symbols")
    lib.axon_start_nrt_profile.argtypes = [
        ctypes.POINTER(ctypes.c_int64),
        ctypes.c_size_t,
    ]
    lib.axon_start_nrt_profile.restype = ctypes.c_int64
    lib.axon_stop_nrt_profile.argtypes = [ctypes.c_char_p]
    lib.axon_stop_nrt_profile.restype = ctypes.c_int64

    @contextlib.contextmanager
    def _hook(output_dir, device_ids):
        import jax

        jax.devices()
        if device_ids:
            ids = (ctypes.c_int64 * len(device_ids))(*device_ids)
            rc = lib.axon_start_nrt_profile(ids, len(device_ids))
        else:
            rc = lib.axon_start_nrt_profile(None, 0)
        if rc != 0:
            raise RuntimeError(f"axon_start_nrt_profile rc={rc}")
        try:
            yield
        finally:
            lib.axon_stop_nrt_profile(str(output_dir).encode())

    mod = types.ModuleType("antenv.axon_hooks")
    mod.get_axon_ntff_profile_hook = lambda: _hook
    mod.set_axon_ntff_profile_hook = lambda h: None
    sys.modules["antenv.axon_hooks"] = mod
    import antenv

    antenv.axon_hooks = mod


# revision 65
# speedup vs baseline: 1.1795x; 1.1795x over previous
"""2-layer GCN (GCNConv -> ReLU -> GCNConv) on 8 TRN2 NeuronCores.

Sharding: output nodes are split into 8 shards (one per core); edges are
partitioned by destination shard so each core owns the scatter-add for its
nodes. Hidden features of source nodes are exchanged with an on-device
AllGather (bf16 table) between the per-shard transform and the aggregation.

Aggregation dataflow (the bottleneck is SWDGE descriptor generation on the
Pool engine at ~8 ns/descriptor, one descriptor per edge):
  - The shared table is AllGathered as TWO half-shard tables (split at
    block 25, via separate zero-offset staging tensors - AP-offset
    collective inputs corrupt data on HW). Each half-table has < 32768
    rows so int16 gather indices cover it directly, and the first half's
    collective fires mid-transform, overlapping the AllGather with the
    rest of the transform and the first gathers.
  - Edges are pre-sorted into 128-dst-node blocks (49 per core, balanced by
    in-degree via host-side relabeling). Blocks are grouped 4 at a time;
    within a group all blocks' half-A chunks come first, then all half-B
    chunks, so gather calls of 8x128 indices run across block boundaries
    and the per-call fixed cost (~1 us) is amortized, while a block's
    chunks stay within a small window of the call stream.
  - Each dma_gather call lands in its own rotating pool tile (NBUF deep);
    a one-hot matrix S (VectorE iota==dmat compare, bf16 out) folds each
    128-edge chunk into the block's PSUM accumulator on TensorE.
  - The table (AllGathered hidden features) is bf16: halves the AllGather
    and the gather read traffic. Layer-1's x@W1 also runs in bf16.
  - Layer-2's transform (transpose + W2 matmul) is fused into layer-1's
    per-block epilogue so PE/DVE work overlaps Pool descriptor generation.
  - GCN_PREP=1 selects an experimental prepare_only+trigger_dma pipeline
    (descriptor gen ahead of the AllGather). With explicit consumer waits
    it is numerically correct but ~1.2 ms slower (per-call prep/trigger
    overhead), so the default stays the direct gather path.
"""

import os

import numpy as np

P = 128
N_CORES = 8
N_NODES = 50000
IN_DIM = 256
HID = 128
NB = 49
SHARD = NB * P  # 6272
NPAD = N_CORES * SHARD  # 50176
# The shared table is AllGathered in two halves (split at block 25 of each
# shard) so each half-table has < 32768 rows and int16 gather indices cover
# it without base tricks, and the first half's collective can fire while
# the second half is still being produced.
HALF_A = 25 * P  # 3200 rows per shard
HALF_B = SHARD - HALF_A  # 3072
NPAD_A = N_CORES * HALF_A  # 25600
NPAD_B = N_CORES * HALF_B  # 24576
CALL_CHUNKS = 8  # 1024 indices per dma_gather call (SWDGE per-call limit)
NG = 96  # max chunk-span constraint for call construction
PREP_W = int(os.environ.get("GCN_PREP_W", "3"))  # prepare-ahead window (calls)

LAST_EXEC_NS = None


def _wrap16(flat, ncols):
    w = np.zeros((16, ncols), np.uint16)
    n = len(flat)
    w[np.arange(n) % 16, np.arange(n) // 16] = flat
    return np.tile(w, (8, 1)).view(np.int16)


def _host_prep(x, edge_index, W1, b1, W2, b2, table_bf16=True):
    import ml_dtypes

    src = np.asarray(edge_index[0], dtype=np.int64)
    dst = np.asarray(edge_index[1], dtype=np.int64)
    x = np.asarray(x, dtype=np.float32)

    indeg = np.bincount(dst, minlength=N_NODES)
    deg = indeg + 1.0
    dinv = (1.0 / np.sqrt(deg)).astype(np.float32)

    # per-shard relabeling: deal nodes (by in-degree desc) round-robin into
    # the 49 dst blocks so block edge counts are balanced across cores.
    old_shard = N_NODES // N_CORES
    new_of_old = np.empty(N_NODES, np.int64)
    old_of_new = np.full(NPAD, -1, np.int64)
    for c in range(N_CORES):
        olds = np.arange(c * old_shard, (c + 1) * old_shard)
        order = olds[np.argsort(-indeg[olds], kind="stable")]
        pos_in_block = np.arange(len(order)) // NB
        block = np.arange(len(order)) % NB
        news = c * SHARD + block * P + pos_in_block
        new_of_old[order] = news
        old_of_new[news] = order

    src_n = new_of_old[src]
    dst_n = new_of_old[dst]

    core_of_dst = dst_n // SHARD
    lo_lists = [[None] * NB for _ in range(N_CORES)]  # half-A edges
    hi_lists = [[None] * NB for _ in range(N_CORES)]  # half-B edges
    for c in range(N_CORES):
        m = core_of_dst == c
        s, d = src_n[m], dst_n[m] - c * SHARD
        b = d // P
        r = d % P
        c_src = s // SHARD
        r_src = s % SHARD
        isb = r_src >= HALF_A
        idx_a = c_src * HALF_A + r_src
        idx_b = c_src * HALF_B + (r_src - HALF_A)
        for bb in range(NB):
            mb = b == bb
            mlo = mb & ~isb
            mhi = mb & isb
            lo_lists[c][bb] = (idx_a[mlo], r[mlo])
            hi_lists[c][bb] = (idx_b[mhi], r[mhi])

    C_lo = np.zeros(NB, np.int64)
    C_hi = np.zeros(NB, np.int64)
    for b in range(NB):
        for c in range(N_CORES):
            C_lo[b] = max(C_lo[b], (len(lo_lists[c][b][0]) + P - 1) // P)
            C_hi[b] = max(C_hi[b], (len(hi_lists[c][b][0]) + P - 1) // P)

    # Chunk layout: blocks grouped GROUPP at a time; within a group all lo
    # chunks first, then all hi chunks. Gather calls merge across blocks
    # within a run (same table base), and a block's chunks stay within a
    # ~GROUPP*17-chunk window so the NG-slot arena ring can't deadlock.
    GROUPP = 4
    lo_start = np.zeros(NB, np.int64)
    hi_start = np.zeros(NB, np.int64)
    regions = []  # (start_chunk, end_chunk, is_hi)
    at = 0
    for g0 in range(0, NB, GROUPP):
        blks = range(g0, min(g0 + GROUPP, NB))
        r0 = at
        for b in blks:
            lo_start[b] = at
            at += int(C_lo[b])
        regions.append((r0, at, False))
        r0 = at
        for b in blks:
            hi_start[b] = at
            at += int(C_hi[b])
        regions.append((r0, at, True))
    NC = at

    idx_mats, d_mats = [], []
    for c in range(N_CORES):
        idx_flat = np.zeros(NC * P, np.int64)
        dloc = np.full((P, NC), -1.0, np.float32)
        for b in range(NB):
            for lists, start in ((lo_lists, lo_start[b]), (hi_lists, hi_start[b])):
                s, r = lists[c][b]
                n = len(s)
                base = int(start) * P
                idx_flat[base : base + n] = s
                j = np.arange(n)
                dloc[j % P, int(start) + j // P] = r
        idx_mats.append(_wrap16(idx_flat, NC * 8))
        d_mats.append(dloc)

    # calls: greedy 8-chunk runs within each region, never wrapping the
    # NG-slot arena mid-call.
    calls = []
    for r0, r1, is_hi in regions:
        if r1 == r0:
            continue
        at = r0
        while at < r1:
            k = min(CALL_CHUNKS, r1 - at, NG - (at % NG))
            calls.append((at, k, is_hi))
            at += k

    xs, dinvs = [], []
    for c in range(N_CORES):
        xc = np.zeros((SHARD, IN_DIM), np.float32)
        dc = np.ones((SHARD,), np.float32)
        sel = old_of_new[c * SHARD : (c + 1) * SHARD]
        real = sel >= 0
        xc[real] = x[sel[real]]
        dc[real] = dinv[sel[real]]
        dw = dc.reshape(NB, P).T.copy()
        xT = np.ascontiguousarray(xc.T.reshape(2, P, SHARD).transpose(1, 0, 2))
        xs.append(xT.reshape(P, 2 * SHARD).astype(ml_dtypes.bfloat16))
        dinvs.append(dw)

    iota = np.tile(np.arange(P, dtype=np.float32)[None, :], (P, 1))
    ident = np.eye(P, dtype=np.float32)
    b1r = np.tile(np.asarray(b1, np.float32)[None, :], (P, 1))
    b2r = np.tile(np.asarray(b2, np.float32)[None, :], (P, 1))

    in_maps = []
    for c in range(N_CORES):
        in_maps.append(
            {
                "x": xs[c],
                "gidx": idx_mats[c],
                "dmat": d_mats[c],
                "dinv": dinvs[c],
                "w1": np.asarray(W1, np.float32).astype(ml_dtypes.bfloat16),
                "w2": np.asarray(W2, np.float32),
                "b1r": b1r,
                "b2r": b2r,
                "iota": iota,
                "ident": ident,
            }
        )

    meta = dict(
        C_lo=C_lo,
        C_hi=C_hi,
        lo_start=lo_start,
        hi_start=hi_start,
        NC=NC,
        calls=calls,
        old_of_new=old_of_new,
    )
    return in_maps, meta


def _build_program(meta, use_prep=True, table_bf16=True):
    import concourse.mybir as mybir
    import concourse.tile as tile
    from concourse import bacc
    from concourse._compat import get_trn_type

    C_lo = meta["C_lo"]
    C_hi = meta["C_hi"]
    lo_start = meta["lo_start"]
    hi_start = meta["hi_start"]
    NC = meta["NC"]
    calls = meta["calls"]
    f32 = mybir.dt.float32
    bf16 = mybir.dt.bfloat16 if table_bf16 else mybir.dt.float32

    scratch = int(os.environ.get("GCN_SCRATCH", "16384"))
    nc = bacc.Bacc(get_trn_type() or "TRN2", dynamic_dma_scratch_size=scratch)
    x_in = nc.dram_tensor("x", [P, 2 * SHARD], mybir.dt.bfloat16, kind="ExternalInput")
    gidx = nc.dram_tensor("gidx", [P, NC * 8], mybir.dt.int16, kind="ExternalInput")
    dmat = nc.dram_tensor("dmat", [P, NC], f32, kind="ExternalInput")
    dinv_in = nc.dram_tensor("dinv", [P, NB], f32, kind="ExternalInput")
    w1_in = nc.dram_tensor("w1", [IN_DIM, HID], mybir.dt.bfloat16, kind="ExternalInput")
    w2_in = nc.dram_tensor("w2", [HID, HID], f32, kind="ExternalInput")
    b1_in = nc.dram_tensor("b1r", [P, HID], f32, kind="ExternalInput")
    b2_in = nc.dram_tensor("b2r", [P, HID], f32, kind="ExternalInput")
    iota_in = nc.dram_tensor("iota", [P, P], f32, kind="ExternalInput")
    ident_in = nc.dram_tensor("ident", [P, P], f32, kind="ExternalInput")
    z_out = nc.dram_tensor("z", [SHARD, HID], f32, kind="ExternalOutput")

    cc1a_in = nc.dram_tensor("cc1a_in", [HALF_A, HID], bf16)
    cc1b_in = nc.dram_tensor("cc1b_in", [HALF_B, HID], bf16)
    table1a = nc.dram_tensor("table1a", [NPAD_A, HID], bf16, addr_space="Shared")
    table1b = nc.dram_tensor("table1b", [NPAD_B, HID], bf16, addr_space="Shared")
    cc2a_in = nc.dram_tensor("cc2a_in", [HALF_A, HID], bf16)
    cc2b_in = nc.dram_tensor("cc2b_in", [HALF_B, HID], bf16)
    table2a = nc.dram_tensor("table2a", [NPAD_A, HID], bf16, addr_space="Shared")
    table2b = nc.dram_tensor("table2b", [NPAD_B, HID], bf16, addr_space="Shared")

    rg = [list(range(N_CORES))]

    NBUF = 14 if table_bf16 else 10
    with tile.TileContext(nc) as tc:
        with (
            tc.tile_pool(name="persist", bufs=1) as pp,
            tc.tile_pool(name="xt", bufs=4) as xtp,
            tc.tile_pool(name="s", bufs=6) as sp,
            tc.tile_pool(name="ep", bufs=4) as ep,
            tc.tile_pool(name="g", bufs=NBUF) as gp,
            tc.tile_pool(name="psum", bufs=2, space="PSUM") as psp,
        ):
            idx_t = pp.tile([P, NC * 8], mybir.dt.int16)
            nc.sync.dma_start(out=idx_t[:], in_=gidx[:])
            dm_t = pp.tile([P, NC], f32)
            nc.sync.dma_start(out=dm_t[:], in_=dmat[:])
            dinv_t = pp.tile([P, NB], f32)
            nc.sync.dma_start(out=dinv_t[:], in_=dinv_in[:])
            iota_t = pp.tile([P, P], f32)
            nc.sync.dma_start(out=iota_t[:], in_=iota_in[:])
            ident_t = pp.tile([P, P], f32)
            nc.sync.dma_start(out=ident_t[:], in_=ident_in[:])
            b1_t = pp.tile([P, HID], f32)
            nc.sync.dma_start(out=b1_t[:], in_=b1_in[:])
            b2_t = pp.tile([P, HID], f32)
            nc.sync.dma_start(out=b2_t[:], in_=b2_in[:])
            w1_t = pp.tile([P, 2 * HID], mybir.dt.bfloat16)
            nc.sync.dma_start(
                out=w1_t[:].rearrange("p (k h) -> p k h", k=2),
                in_=w1_in[:].rearrange("(k p) h -> p k h", p=P),
            )
            w2_t = pp.tile([P, HID], f32)
            nc.sync.dma_start(out=w2_t[:], in_=w2_in[:])

            hbuf = pp.tile([P, SHARD], f32)
            h2buf = pp.tile([P, SHARD], f32)

            dma_sem = nc.alloc_semaphore("gather_dma")

            # chunk -> (call index, call start chunk)
            call_of_chunk = {}
            for j, (at, k, is_hi) in enumerate(calls):
                for c in range(at, at + k):
                    call_of_chunk[c] = (j, at)

            gtiles = {}
            fired_base = [0]  # calls fired in earlier layers

            def emit_preps(tab_a, tab_b, prep, j0=0, j1=None):
                for j, (at, k, is_b) in list(enumerate(calls))[j0:j1]:
                    n = k * P
                    src = tab_b[:, :] if is_b else tab_a[:, :]
                    G = gp.tile([P, CALL_CHUNKS * P], bf16, tag="g")
                    gtiles[j] = G[:].rearrange("p (c d) -> p c d", d=P)
                    kwargs = dict(prepare_only=True, sem=dma_sem) if prep else {}
                    nc.gpsimd.dma_gather(
                        gtiles[j][:, 0:k, :],
                        src,
                        idx_t[:, at * 8 : (at + k) * 8],
                        n,
                        n,
                        HID,
                        **kwargs,
                    )
                    if prep and j >= PREP_W - 1:
                        nc.gpsimd.trigger_dma(count=None)
                if prep and len(calls) < PREP_W:
                    nc.gpsimd.trigger_dma(count=None)

            def transform2_block(b, x2_tile):
                # fused layer-2 transform for dst block b: h2 = dinv*(x2 @ W2)
                tp = psp.tile([P, P], f32, tag="tp")
                nc.tensor.transpose(out=tp[:], in_=x2_tile, identity=ident_t[:])
                xT = xtp.tile([P, P], f32, tag="xT")
                nc.scalar.copy(out=xT[:], in_=tp[:])
                hp = psp.tile([P, HID], f32, tag="hp2")
                nc.tensor.matmul(
                    out=hp[:], lhsT=xT[:], rhs=w2_t[:], start=True, stop=True
                )
                sl = h2buf[:, b * P : (b + 1) * P]
                nc.vector.tensor_scalar(
                    out=sl, in0=hp[:], scalar1=dinv_t[:, b : b + 1],
                    scalar2=None, op0=mybir.AluOpType.mult,
                )
                hc = xtp.tile([P, HID], bf16, tag="hc2")
                nc.scalar.copy(out=hc[:], in_=sl)
                if b < HALF_A // P:
                    nc.sync.dma_start(
                        out=cc2a_in[b * P : (b + 1) * P, :], in_=hc[:]
                    )
                else:
                    bo = b - HALF_A // P
                    nc.sync.dma_start(
                        out=cc2b_in[bo * P : (bo + 1) * P, :], in_=hc[:]
                    )

            fuse = bool(int(os.environ.get("GCN_FUSE", "1")))
            x2buf = None if fuse else pp.tile([P, SHARD], f32)

            def consume(hsrc, bias_t, relu, z_dram, use_prep_waits=False,
                        b0=0, b1=NB):
                maxj = [-1]

                def need_call(j):
                    # prep path: Tile's consumer waits don't cover the
                    # deferred DMA; wait explicitly on the shared DMA sem.
                    # Completions are FIFO, so >=16*(global call idx+1)
                    # implies all earlier calls landed too.
                    if use_prep_waits and j > maxj[0]:
                        nc.tensor.wait_ge(
                            dma_sem, 16 * (fired_base[0] + j + 1)
                        )
                        maxj[0] = j

                for b in range(b0, b1):
                    cl = [int(lo_start[b]) + i for i in range(int(C_lo[b]))] + [
                        int(hi_start[b]) + i for i in range(int(C_hi[b]))
                    ]
                    acc = psp.tile([P, HID], f32, tag="acc")
                    for i, c in enumerate(cl):
                        S = sp.tile([P, P], bf16, tag="S")
                        nc.vector.tensor_tensor(
                            out=S[:], in0=iota_t[:],
                            in1=dm_t[:, c : c + 1].to_broadcast([P, P]),
                            op=mybir.AluOpType.is_equal,
                        )
                        j, at = call_of_chunk[c]
                        need_call(j)
                        nc.tensor.matmul(
                            out=acc[:], lhsT=S[:], rhs=gtiles[j][:, c - at, :],
                            start=(i == 0), stop=(i == len(cl) - 1),
                        )
                    t1 = ep.tile([P, HID], f32, tag="t1")
                    nc.vector.tensor_tensor(
                        out=t1[:], in0=acc[:],
                        in1=hsrc[:, b * P : (b + 1) * P],
                        op=mybir.AluOpType.add,
                    )
                    t2 = ep.tile([P, HID], f32, tag="t2")
                    nc.vector.scalar_tensor_tensor(
                        out=t2[:], in0=t1[:],
                        scalar=dinv_t[:, b : b + 1], in1=bias_t[:],
                        op0=mybir.AluOpType.mult, op1=mybir.AluOpType.add,
                    )
                    if relu:
                        if fuse:
                            x2 = ep.tile([P, HID], f32, tag="x2")
                            nc.scalar.activation(
                                out=x2[:], in_=t2[:],
                                func=mybir.ActivationFunctionType.Relu,
                            )
                            transform2_block(b, x2[:])
                        else:
                            nc.scalar.activation(
                                out=x2buf[:, b * P : (b + 1) * P], in_=t2[:],
                                func=mybir.ActivationFunctionType.Relu,
                            )
                    else:
                        nc.sync.dma_start(
                            out=z_dram[b * P : (b + 1) * P, :], in_=t2[:]
                        )

            # layer 1 transform: h1 = dinv * (x @ W1), bf16 inputs
            xT_t = pp.tile([P, 2 * SHARD], mybir.dt.bfloat16)
            nc.sync.dma_start(out=xT_t[:], in_=x_in[:])
            xT3 = xT_t[:].rearrange("p (k n) -> p k n", k=2)
            for t in range(NB):
                hp = psp.tile([P, HID], f32, tag="hp")
                for k in range(2):
                    nc.tensor.matmul(
                        out=hp[:], lhsT=xT3[:, k, t * P : (t + 1) * P],
                        rhs=w1_t[:, k * HID : (k + 1) * HID],
                        start=(k == 0), stop=(k == 1),
                    )
                sl = hbuf[:, t * P : (t + 1) * P]
                nc.vector.tensor_scalar(
                    out=sl, in0=hp[:], scalar1=dinv_t[:, t : t + 1],
                    scalar2=None, op0=mybir.AluOpType.mult,
                )
                hc = xtp.tile([P, HID], bf16, tag="hc1")
                nc.scalar.copy(out=hc[:], in_=sl)
                if t < HALF_A // P:
                    nc.sync.dma_start(
                        out=cc1a_in[t * P : (t + 1) * P, :], in_=hc[:]
                    )
                else:
                    to = t - HALF_A // P
                    nc.sync.dma_start(
                        out=cc1b_in[to * P : (to + 1) * P, :], in_=hc[:]
                    )
                if t == HALF_A // P - 1:
                    # first-half table ships while the rest is produced
                    nc.gpsimd.collective_compute(
                        "AllGather", mybir.AluOpType.bypass, replica_groups=rg,
                        ins=[cc1a_in[:]], outs=[table1a[:]],
                    )

            nc.gpsimd.collective_compute(
                "AllGather", mybir.AluOpType.bypass, replica_groups=rg,
                ins=[cc1b_in[:]], outs=[table1b[:]],
            )
            if fuse and not use_prep:
                # Interleaved emission: cc2a sits mid-way through layer-1's
                # Pool gather stream (after group 8's calls, once consume of
                # blocks 0..27 has produced cc2a_in), so AllGather 2a runs
                # under the remaining layer-1 descriptor generation.
                split_call = next(
                    j for j, (at, k, ib) in enumerate(calls)
                    if at >= int(lo_start[36])
                )
                emit_preps(table1a, table1b, False, 0, split_call)
                consume(hbuf, b1_t, True, None, b0=0, b1=28)
                nc.gpsimd.collective_compute(
                    "AllGather", mybir.AluOpType.bypass, replica_groups=rg,
                    ins=[cc2a_in[:]], outs=[table2a[:]],
                )
                emit_preps(table1a, table1b, False, split_call, None)
                consume(hbuf, b1_t, True, None, b0=28, b1=NB)
            else:
                emit_preps(table1a, table1b, use_prep)
                consume(hbuf, b1_t, True, None, use_prep_waits=use_prep)
                if use_prep:
                    fired_base[0] += len(calls)
                if not fuse:
                    for b in range(NB):
                        transform2_block(b, x2buf[:, b * P : (b + 1) * P])
                nc.gpsimd.collective_compute(
                    "AllGather", mybir.AluOpType.bypass, replica_groups=rg,
                    ins=[cc2a_in[:]], outs=[table2a[:]],
                )

            nc.gpsimd.collective_compute(
                "AllGather", mybir.AluOpType.bypass, replica_groups=rg,
                ins=[cc2b_in[:]], outs=[table2b[:]],
            )
            emit_preps(table2a, table2b, use_prep)
            consume(h2buf, b2_t, False, z_out, use_prep_waits=use_prep)

    nc.compile()
    return nc


def kernel(x, edge_index, W1, b1, W2, b2):
    global LAST_EXEC_NS
    from concourse.bass_utils import run_bass_kernel_spmd

    trace = bool(int(os.environ.get("GCN_TRACE", "0")))
    if trace:
        try:  # NTFF profiling shim (axon images lack antenv.axon_hooks)
            _install_ntff_shim()
        except Exception:
            trace = False

    use_prep = bool(int(os.environ.get("GCN_PREP", "0")))
    table_bf16 = bool(int(os.environ.get("GCN_BF16", "1")))
    in_maps, meta = _host_prep(x, edge_index, W1, b1, W2, b2, table_bf16=table_bf16)
    nc = _build_program(meta, use_prep=use_prep, table_bf16=table_bf16)
    res = run_bass_kernel_spmd(
        nc, in_maps, core_ids=list(range(N_CORES)), trace=trace
    )
    LAST_EXEC_NS = res.exec_time_ns

    old_of_new = meta["old_of_new"]
    z = np.zeros((N_NODES, HID), np.float32)
    for c in range(N_CORES):
        zc = np.asarray(res.results[c]["z"])
        sel = old_of_new[c * SHARD : (c + 1) * SHARD]
        real = sel >= 0
        z[sel[real]] = zc[real]
    return z


def _install_ntff_shim():
    import contextlib
    import ctypes
    import sys
    import types

    if "antenv.axon_hooks" in sys.modules:
        return
    lib = ctypes.CDLL("/opt/axon/libaxon_pjrt.so")
    if not hasattr(lib, "axon_start_nrt_profile"):
        raise RuntimeError("no profile symbols")
    lib.axon_start_nrt_profile.argtypes = [
        ctypes.POINTER(ctypes.c_int64),
        ctypes.c_size_t,
    ]
    lib.axon_start_nrt_profile.restype = ctypes.c_int64
    lib.axon_stop_nrt_profile.argtypes = [ctypes.c_char_p]
    lib.axon_stop_nrt_profile.restype = ctypes.c_int64

    @contextlib.contextmanager
    def _hook(output_dir, device_ids):
        import jax

        jax.devices()
        if device_ids:
            ids = (ctypes.c_int64 * len(device_ids))(*device_ids)
            rc = lib.axon_start_nrt_profile(ids, len(device_ids))
        else:
            rc = lib.axon_start_nrt_profile(None, 0)
        if rc != 0:
            raise RuntimeError(f"axon_start_nrt_profile rc={rc}")
        try:
            yield
        finally:
            lib.axon_stop_nrt_profile(str(output_dir).encode())

    mod = types.ModuleType("antenv.axon_hooks")
    mod.get_axon_ntff_profile_hook = lambda: _hook
    mod.set_axon_ntff_profile_hook = lambda h: None
    sys.modules["antenv.axon_hooks"] = mod
    import antenv

    antenv.axon_hooks = mod
